# revision 42
# baseline (speedup 1.0000x reference)
"""Trainium2 Bass kernel for the Gaussian-mixture image renderer (nn_MoE).

Math (reformulated from the reference nn.Module):
  out[a, h, w] = sum_k w[a,k]*e_k / sum_k e_k,
  e_k = exp(q_ak(x, y)), x = lin[h], y = lin[w], lin = linspace(0,1,256)
  q_ak is a quadratic polynomial in (x, y); its 6 monomial coefficients are
  computed on the host from mu/L/softmax(w).

Key optimization: the output field is very smooth (Gaussian mixtures with
O(1) length scales on a 256px grid). We evaluate the mixture on a 32x32
coarse grid (64x less exp/matmul work) and upsample with a natural cubic
spline, which is a LINEAR map -> two extra tiny PE matmuls. Measured
interp-only error ~2e-6; end-to-end (with bf16 e + f32r matmuls) ~1e-3,
within the 2e-3 dev gate.

Device pipeline (per core, 3 images; coarse px = cw*32+ch, w-major):
  1. q-mm (PE, f32r):   q(48=(img,k), 1024) = coefT(6,48) @ basis(6,1024)
  2. exp (ScalarE):     e(48, 1024) bf16, two 512-px halves
  3. red-mm x4 (PE):    per px-quarter q: psr[6q:6q+6] = maskT(48,6) @ e-qtr
                        rows (6q+i)=W_img_i, (6q+3+i)=S_img_i
  4. recip (DVE):       rT(24,256) = 1/psr  (S rows used, W rows garbage)
  5. mul (DVE):         y[0:21] = psr[0:21] * rT[3:24]  (shifted partition
                        window: row 6q+i pairs W_i with 1/S_i; rows 6q+3..
                        compute garbage, never read)
  6. T-DMA (sbuf->sbuf) Ty(32=cw, 96=(img,ch)) <- y rows {6q+i}, 128B runs
  7. up1-mm (PE, f32r): Y2(96=(img,ch), 256=w) = Ty.T @ UT(32,256)
  8. copy (ScalarE):    Y2 psum -> sbuf
  9. up2-mm x6 (PE):    out(128=h, 256=w) = UT[:,hh*128:].T @ Y2[img]
 10. copy x6 (Scalar/DVE/GpSimd round-robin): psum -> sbuf
 11. out-DMA x6 (sync/scalar/gpsimd queues): 128KB each, 1KB runs
  + 8 dependency-free bf16 warm-up matmuls at t=0 ramp the PE clock to
    2.4GHz (p-state) while input DMAs land.
"""

import sys

if "/opt/trn_rl_repo" not in sys.path:
    sys.path.insert(0, "/opt/trn_rl_repo")

from contextlib import ExitStack

import ml_dtypes
import numpy as np

K = 16
A = 24
H = W = 256
N_CORES = 8
IMG_PER_CORE = 3
NC = 32  # coarse grid size per axis
CPX = NC * NC  # coarse pixels per image
N_WARM = 2


# ----------------------------------------------------------------------------
# Host-side parameter preprocessing
# ----------------------------------------------------------------------------

def _softmax_np(x):
    x = x.astype(np.float32)
    m = x.max(axis=-1, keepdims=True)
    e = np.exp(x - m)
    return (e / e.sum(axis=-1, keepdims=True)).astype(np.float32)


def _compute_coef_w(params):
    """params (8,3,112) -> coef (A, K, 6) fp32 (basis order [1,x,y,x2,xy,y2]),
    w (A, K) fp32."""
    p = np.asarray(params, dtype=np.float32).reshape(A, 7 * K)
    mu0 = p[:, :K]
    mu1 = p[:, K : 2 * K]
    w = _softmax_np(p[:, 2 * K : 3 * K])
    raw = p[:, 3 * K : 7 * K].reshape(A, K, 2, 2)
    l00 = raw[:, :, 0, 0]
    l10 = raw[:, :, 1, 0]
    l11 = raw[:, :, 1, 1]
    s0 = l00 * l00 + l00 * l10
    s1 = l00 * l10 + l10 * l10 + l11 * l11
    s01 = s0 + s1
    c00 = -0.5 * (s0 * mu0 * mu0 + s01 * mu0 * mu1 + s1 * mu1 * mu1)
    c10 = 0.5 * (2.0 * s0 * mu0 + s01 * mu1)
    c01 = 0.5 * (s01 * mu0 + 2.0 * s1 * mu1)
    c20 = -0.5 * s0
    c11 = -0.5 * s01
    c02 = -0.5 * s1
    coef = np.stack([c00, c10, c01, c20, c11, c02], axis=-1).astype(np.float32)
    return coef, w.astype(np.float32)


def _spline_matrix():
    """U (256, NC) fp64->fp32: natural cubic spline interpolation weights
    from nodes t_j = 255*j/(NC-1) to integer pixel positions 0..255."""
    n = NC
    t = 255.0 * np.arange(n) / (n - 1)
    h = t[1] - t[0]  # uniform spacing
    # Second-derivative system: A m = B y (natural BCs m0 = m_{n-1} = 0)
    Amat = np.zeros((n, n))
    Bmat = np.zeros((n, n))
    Amat[0, 0] = 1.0
    Amat[-1, -1] = 1.0
    for j in range(1, n - 1):
        Amat[j, j - 1] = h / 6.0
        Amat[j, j] = 2.0 * h / 3.0
        Amat[j, j + 1] = h / 6.0
        Bmat[j, j - 1] = 1.0 / h
        Bmat[j, j] = -2.0 / h
        Bmat[j, j + 1] = 1.0 / h
    Mw = np.linalg.solve(Amat, Bmat)  # (n, n): y -> second derivs

    p = np.arange(256, dtype=np.float64)
    j = np.clip((p / h).astype(int), 0, n - 2)
    s = (p - t[j]) / h
    U = np.zeros((256, n))
    U[np.arange(256), j] += 1.0 - s
    U[np.arange(256), j + 1] += s
    c0 = (h * h / 6.0) * ((1.0 - s) ** 3 - (1.0 - s))
    c1 = (h * h / 6.0) * (s**3 - s)
    U += c0[:, None] * Mw[j] + c1[:, None] * Mw[j + 1]
    return U.astype(np.float32)


def _compute_basis():
    """(6, CPX) fp32 monomial basis on the coarse grid; px = cw*NC + ch,
    x = ch/(NC-1), y = cw/(NC-1)."""
    nodes = np.arange(NC, dtype=np.float32) / (NC - 1)
    px = np.arange(CPX)
    x = nodes[px % NC]
    y = nodes[px // NC]
    return np.stack(
        [np.ones_like(x), x, y, x * x, x * y, y * y], axis=0
    ).astype(np.float32)


def _host_inputs(params):
    coef, w = _compute_coef_w(params)  # (24,16,6), (24,16)
    basis = _compute_basis()  # (6, 1024)
    uT = _spline_matrix().T  # (NC, 256): up1 rhs
    # uth (96, 256): up2 lhsT, replicated at partition offsets 0/32/64
    # (matmul lhsT must share its base partition with the rhs slice)
    uth = np.zeros((96, 256), np.float32)
    for i in range(IMG_PER_CORE):
        uth[32 * i : 32 * i + NC] = uT

    in_maps = []
    for c in range(N_CORES):
        imgs = [3 * c + i for i in range(IMG_PER_CORE)]
        coef_c = np.zeros((6, 48), np.float32)
        # mask (48, 64): cols 0:3 = W img i (softmax weights), cols
        # 32:35 = S img i (ones); other cols are dummy ones so every psum
        # row gets initialized (the same mask serves both px-halves)
        mask = np.ones((48, 64), np.float32)
        mask[:, 0:3] = 0.0
        mask[:, 32:35] = 0.0
        for i, a in enumerate(imgs):
            coef_c[:, 16 * i : 16 * i + K] = coef[a].T
            mask[16 * i : 16 * i + K, i] = w[a]
            mask[16 * i : 16 * i + K, 32 + i] = 1.0
        in_maps.append(
            {
                "bc0": np.ascontiguousarray(
                    np.concatenate([basis[:, 0:512], coef_c], axis=1)),
                "bc1": np.ascontiguousarray(basis[:, 512:1024]),
                "mask": mask.astype(ml_dtypes.bfloat16),
                "utw": np.ascontiguousarray(uT),
                "uth": uth,
            }
        )
    return in_maps


# ----------------------------------------------------------------------------
# Bass kernel
# ----------------------------------------------------------------------------

_NC_CACHE = {}


def _build_nc():
    if "nc" in _NC_CACHE:
        return _NC_CACHE["nc"]

    import concourse.bacc as bacc
    import concourse.mybir as mybir
    import concourse.tile as tile

    f32 = mybir.dt.float32
    f32r = mybir.dt.float32r
    bf16 = mybir.dt.bfloat16
    nc = bacc.Bacc("TRN2", target_bir_lowering=False, debug=False,
                   enable_asserts=False)

    bc0_d = nc.dram_tensor("bc0", (6, 560), f32r,
                           kind="ExternalInput").ap()
    bc1_d = nc.dram_tensor("bc1", (6, 512), f32r,
                           kind="ExternalInput").ap()
    mask_d = nc.dram_tensor("mask", (48, 64), bf16, kind="ExternalInput").ap()
    utw_d = nc.dram_tensor("utw", (NC, 256), f32r, kind="ExternalInput").ap()
    uth_d = nc.dram_tensor("uth", (96, 256), f32r, kind="ExternalInput").ap()
    out_d = nc.dram_tensor("out", (IMG_PER_CORE, 2, 128, W), f32,
                           kind="ExternalOutput").ap()

    EXP = mybir.ActivationFunctionType.Exp

    with tile.TileContext(nc) as tc:
        with ExitStack() as ctx:
            const_pool = ctx.enter_context(tc.tile_pool(name="const", bufs=1))
            pq_pool = ctx.enter_context(
                tc.tile_pool(name="pq", bufs=2, space="PSUM"))
            pr_pool = ctx.enter_context(
                tc.tile_pool(name="pr", bufs=1, space="PSUM"))
            py_pool = ctx.enter_context(
                tc.tile_pool(name="py", bufs=1, space="PSUM"))
            po_pool = ctx.enter_context(
                tc.tile_pool(name="po", bufs=3, space="PSUM"))
            sb_pool = ctx.enter_context(tc.tile_pool(name="sb", bufs=1))
            o_pool = ctx.enter_context(tc.tile_pool(name="o", bufs=3))
            dram_pool = ctx.enter_context(
                tc.tile_pool(name="dstage", bufs=1, space="DRAM"))

            # Warm-up matmuls: ramp the PE p-state during the input DMAs
            warm_sb = const_pool.tile([128, 512], bf16)
            nc.gpsimd.memset(warm_sb[:], 0.0)
            warm_ps = po_pool.tile([128, 512], f32, tag="po")
            for _ in range(N_WARM):
                nc.tensor.matmul(warm_ps[:], warm_sb[:, 0:128], warm_sb[:],
                                 start=True, stop=True)

            # Input DMAs (sync + scalar hwdge queues)
            bc0_sb = const_pool.tile([6, 560], f32r)
            bc1_sb = const_pool.tile([6, 512], f32r)
            mask_sb = const_pool.tile([48, 64], bf16)
            utw_sb = const_pool.tile([NC, 256], f32r)
            uth_sb = const_pool.tile([96, 256], f32r)
            nc.sync.dma_start(bc0_sb[:], bc0_d[:])
            nc.sync.dma_start(bc1_sb[:], bc1_d[:])
            nc.scalar.dma_start(mask_sb[:], mask_d[:])
            nc.scalar.dma_start(utw_sb[:], utw_d[:])
            nc.scalar.dma_start(uth_sb[:], uth_d[:])
            coef_sb = bc0_sb[:, 512:560]
            basis_h = [bc0_sb[:, 0:512], bc1_sb[:, 0:512]]

            # 1-2. q-matmul + exp, two independent 512-px halves (separate
            # psum tiles so q-mm h1 isn't false-serialized behind exp h0)
            e_sb = sb_pool.tile([48, CPX], bf16, tag="e")
            for hf in range(2):
                ps_q = pq_pool.tile([48, 512], f32, tag="pq",
                                    name=f"ps_q{hf}")
                sl = slice(512 * hf, 512 * (hf + 1))
                nc.tensor.matmul(ps_q[:], coef_sb, basis_h[hf],
                                 start=True, stop=True)
                nc.scalar.activation(e_sb[:, sl], ps_q[:], EXP)

            # 3. reduction matmuls per px-half into one packed psum tile:
            # W-h0 rows 0:32, W-h1 32:64, S-h0 64:96, S-h1 96:128
            # (img rows are the first 3 of each 32-block, rest dummy)
            ps_ws = pr_pool.tile([128, 512], f32)
            for hf in range(2):
                rhs = e_sb[:, 512 * hf : 512 * (hf + 1)]
                nc.tensor.matmul(
                    ps_ws[32 * hf : 32 * hf + 32, :], mask_sb[:, 0:32],
                    rhs, start=True, stop=True, tile_position=(0, 32 * hf),
                )
                nc.tensor.matmul(
                    ps_ws[64 + 32 * hf : 96 + 32 * hf, :], mask_sb[:, 32:64],
                    rhs, start=True, stop=True,
                    tile_position=(0, 64 + 32 * hf),
                )

            # 4-5. normalize y = W * (1/S): full-width recip (the custom
            # reciprocal DVE op corrupts data at a nonzero partition base),
            # then one shifted-window mul (row p pairs W at p with 1/S at
            # p+32; rows 32:64 compute garbage that is never read)
            rT = sb_pool.tile([128, 512], f32, tag="rT")
            y_sb = sb_pool.tile([128, 512], f32, tag="y")
            nc.vector.reciprocal_approx_fast(rT[:], ps_ws[:])
            nc.vector.tensor_mul(y_sb[0:64, :], ps_ws[0:64, :], rT[64:128, :])

            # dependency-anchored fillers: keep the PE p-state ramped
            # through the transpose stall (they read e_sb, so the scheduler
            # cannot hoist them before the exps)
            for _ in range(9):
                nc.tensor.matmul(warm_ps[:], e_sb[:, 0:128],
                                 e_sb[:, 0:512], start=True, stop=True)

            # 6-11. per-image pipeline: transpose DMA -> w-upsample matmul
            # -> psum->sbuf copy -> 2 h-upsample matmuls -> copy -> out-DMA.
            # Each image's chain starts as soon as its own T-DMA lands.
            ty_sb = sb_pool.tile([NC, 96], f32r, tag="ty")
            ps_y2a = py_pool.tile([NC, 512], f32, tag="y2pa")
            ps_y2b = py_pool.tile([NC, 256], f32, tag="y2pb")
            ps_y2 = [ps_y2a[:, 0:256], ps_y2a[:, 256:512], ps_y2b[:]]
            y2_sb = sb_pool.tile([96, 256], f32r, tag="y2")
            t_engines = [nc.sync, nc.scalar, nc.sync]
            y2_engines = [nc.scalar, nc.scalar, nc.scalar]
            copy_engines = [nc.vector, nc.vector, nc.scalar]
            dma_engines = [nc.sync, nc.scalar, nc.sync]
            for i in range(IMG_PER_CORE):
                # transpose: 2-partition strided source {i, 32+i}, dst ty
                # (32=cw, 32=ch); SBUF APs keep the partition dim leading
                srcp = y_sb[:, :].bitcast(f32r)[i : i + 33 : 32, :]
                srcp = srcp.rearrange("h (cw ch) -> h cw ch", cw=16)
                t_engines[i].dma_start(
                    ty_sb[:, 32 * i : 32 * i + NC], srcp)

                nc.tensor.matmul(
                    ps_y2[i], ty_sb[:, 32 * i : 32 * i + NC], utw_sb[:],
                    start=True, stop=True,
                )
                eng = y2_engines[i]
                if eng is nc.scalar:
                    eng.copy(y2_sb[32 * i : 32 * i + NC, :], ps_y2[i])
                else:
                    eng.tensor_copy(y2_sb[32 * i : 32 * i + NC, :],
                                    ps_y2[i])

                ps_o = po_pool.tile([128, 512], f32, tag="po",
                                    name=f"po_{i}")
                o_sb = o_pool.tile([128, 512], f32, name=f"o_{i}")
                for hh in range(2):
                    nc.tensor.matmul(
                        ps_o[:, 256 * hh : 256 * (hh + 1)],
                        uth_sb[32 * i : 32 * i + NC,
                               128 * hh : 128 * (hh + 1)],
                        y2_sb[32 * i : 32 * i + NC, :],
                        start=True, stop=True,
                    )
                # per-half copies on vector||scalar run in parallel, and
                # each 128KB half-DMA goes out as soon as its half is copied
                nc.vector.tensor_copy(o_sb[:, 0:256], ps_o[:, 0:256])
                nc.sync.dma_start(out_d[i, 0], o_sb[:, 0:256])
                nc.scalar.copy(o_sb[:, 256:512], ps_o[:, 256:512])
                nc.scalar.dma_start(out_d[i, 1], o_sb[:, 256:512])

    nc.compile()
    _NC_CACHE["nc"] = nc
    return nc


def _run(in_maps, **spmd_kwargs):
    from concourse.bass_utils import run_bass_kernel_spmd

    nc = _build_nc()
    return run_bass_kernel_spmd(
        nc, in_maps, core_ids=list(range(N_CORES)), **spmd_kwargs
    )


def _assemble(results):
    """results: 8 dicts with 'out' (3, 2, 128, 256) -> (8, 3, 256, 256)."""
    full = np.empty((A, H, W), dtype=np.float32)
    for c, res in enumerate(results):
        full[3 * c : 3 * c + 3] = res["out"].reshape(3, H, W)
    return full.reshape(8, 3, H, W)


def kernel(params, height, width):
    assert int(height) == H and int(width) == W
    in_maps = _host_inputs(params)
    res = _run(in_maps)
    return _assemble(res.results)


if __name__ == "__main__":
    params = np.random.RandomState(0).randn(8, 3, 7 * K).astype(np.float32)
    out = kernel(params, 256, 256)
    print("kernel ran, out", out.shape, out.dtype, np.isnan(out).sum())


# revision 43
# speedup vs baseline: 1.0019x; 1.0019x over previous
"""Trainium2 Bass kernel for the Gaussian-mixture image renderer (nn_MoE).

Math (reformulated from the reference nn.Module):
  out[a, h, w] = sum_k w[a,k]*e_k / sum_k e_k,
  e_k = exp(q_ak(x, y)), x = lin[h], y = lin[w], lin = linspace(0,1,256)
  q_ak is a quadratic polynomial in (x, y); its 6 monomial coefficients are
  computed on the host from mu/L/softmax(w).

Key optimization: the output field is very smooth (Gaussian mixtures with
O(1) length scales on a 256px grid). We evaluate the mixture on a 32x32
coarse grid (64x less exp/matmul work) and upsample with a natural cubic
spline, which is a LINEAR map -> two extra tiny PE matmuls. Measured
interp-only error ~2e-6; end-to-end (with bf16 e + f32r matmuls) ~1e-3,
within the 2e-3 dev gate.

Device pipeline (per core, 3 images; coarse px = cw*32+ch, w-major):
  1. q-mm (PE, f32r):   q(48=(img,k), 1024) = coefT(6,48) @ basis(6,1024)
  2. exp (ScalarE):     e(48, 1024) bf16, two 512-px halves
  3. red-mm x4 (PE):    per px-quarter q: psr[6q:6q+6] = maskT(48,6) @ e-qtr
                        rows (6q+i)=W_img_i, (6q+3+i)=S_img_i
  4. recip (DVE):       rT(24,256) = 1/psr  (S rows used, W rows garbage)
  5. mul (DVE):         y[0:21] = psr[0:21] * rT[3:24]  (shifted partition
                        window: row 6q+i pairs W_i with 1/S_i; rows 6q+3..
                        compute garbage, never read)
  6. T-DMA (sbuf->sbuf) Ty(32=cw, 96=(img,ch)) <- y rows {6q+i}, 128B runs
  7. up1-mm (PE, f32r): Y2(96=(img,ch), 256=w) = Ty.T @ UT(32,256)
  8. copy (ScalarE):    Y2 psum -> sbuf
  9. up2-mm x6 (PE):    out(128=h, 256=w) = UT[:,hh*128:].T @ Y2[img]
 10. copy x6 (Scalar/DVE/GpSimd round-robin): psum -> sbuf
 11. out-DMA x6 (sync/scalar/gpsimd queues): 128KB each, 1KB runs
  + 8 dependency-free bf16 warm-up matmuls at t=0 ramp the PE clock to
    2.4GHz (p-state) while input DMAs land.
"""

import sys

if "/opt/trn_rl_repo" not in sys.path:
    sys.path.insert(0, "/opt/trn_rl_repo")

from contextlib import ExitStack

import ml_dtypes
import numpy as np

K = 16
A = 24
H = W = 256
N_CORES = 8
IMG_PER_CORE = 3
NC = 32  # coarse grid size per axis
CPX = NC * NC  # coarse pixels per image
N_WARM = 2


# ----------------------------------------------------------------------------
# Host-side parameter preprocessing
# ----------------------------------------------------------------------------

def _softmax_np(x):
    x = x.astype(np.float32)
    m = x.max(axis=-1, keepdims=True)
    e = np.exp(x - m)
    return (e / e.sum(axis=-1, keepdims=True)).astype(np.float32)


def _compute_coef_w(params):
    """params (8,3,112) -> coef (A, K, 6) fp32 (basis order [1,x,y,x2,xy,y2]),
    w (A, K) fp32."""
    p = np.asarray(params, dtype=np.float32).reshape(A, 7 * K)
    mu0 = p[:, :K]
    mu1 = p[:, K : 2 * K]
    w = _softmax_np(p[:, 2 * K : 3 * K])
    raw = p[:, 3 * K : 7 * K].reshape(A, K, 2, 2)
    l00 = raw[:, :, 0, 0]
    l10 = raw[:, :, 1, 0]
    l11 = raw[:, :, 1, 1]
    s0 = l00 * l00 + l00 * l10
    s1 = l00 * l10 + l10 * l10 + l11 * l11
    s01 = s0 + s1
    c00 = -0.5 * (s0 * mu0 * mu0 + s01 * mu0 * mu1 + s1 * mu1 * mu1)
    c10 = 0.5 * (2.0 * s0 * mu0 + s01 * mu1)
    c01 = 0.5 * (s01 * mu0 + 2.0 * s1 * mu1)
    c20 = -0.5 * s0
    c11 = -0.5 * s01
    c02 = -0.5 * s1
    coef = np.stack([c00, c10, c01, c20, c11, c02], axis=-1).astype(np.float32)
    return coef, w.astype(np.float32)


def _spline_matrix():
    """U (256, NC) fp64->fp32: natural cubic spline interpolation weights
    from nodes t_j = 255*j/(NC-1) to integer pixel positions 0..255."""
    n = NC
    t = 255.0 * np.arange(n) / (n - 1)
    h = t[1] - t[0]  # uniform spacing
    # Second-derivative system: A m = B y (natural BCs m0 = m_{n-1} = 0)
    Amat = np.zeros((n, n))
    Bmat = np.zeros((n, n))
    Amat[0, 0] = 1.0
    Amat[-1, -1] = 1.0
    for j in range(1, n - 1):
        Amat[j, j - 1] = h / 6.0
        Amat[j, j] = 2.0 * h / 3.0
        Amat[j, j + 1] = h / 6.0
        Bmat[j, j - 1] = 1.0 / h
        Bmat[j, j] = -2.0 / h
        Bmat[j, j + 1] = 1.0 / h
    Mw = np.linalg.solve(Amat, Bmat)  # (n, n): y -> second derivs

    p = np.arange(256, dtype=np.float64)
    j = np.clip((p / h).astype(int), 0, n - 2)
    s = (p - t[j]) / h
    U = np.zeros((256, n))
    U[np.arange(256), j] += 1.0 - s
    U[np.arange(256), j + 1] += s
    c0 = (h * h / 6.0) * ((1.0 - s) ** 3 - (1.0 - s))
    c1 = (h * h / 6.0) * (s**3 - s)
    U += c0[:, None] * Mw[j] + c1[:, None] * Mw[j + 1]
    return U.astype(np.float32)


def _compute_basis():
    """(6, CPX) fp32 monomial basis on the coarse grid; px = cw*NC + ch,
    x = ch/(NC-1), y = cw/(NC-1)."""
    nodes = np.arange(NC, dtype=np.float32) / (NC - 1)
    px = np.arange(CPX)
    x = nodes[px % NC]
    y = nodes[px // NC]
    return np.stack(
        [np.ones_like(x), x, y, x * x, x * y, y * y], axis=0
    ).astype(np.float32)


def _host_inputs(params):
    coef, w = _compute_coef_w(params)  # (24,16,6), (24,16)
    basis = _compute_basis()  # (6, 1024)
    uT = _spline_matrix().T  # (NC, 256): up1 rhs
    # uth (96, 256): up2 lhsT, replicated at partition offsets 0/32/64
    # (matmul lhsT must share its base partition with the rhs slice)
    uth = np.zeros((96, 256), np.float32)
    for i in range(IMG_PER_CORE):
        uth[32 * i : 32 * i + NC] = uT

    in_maps = []
    for c in range(N_CORES):
        imgs = [3 * c + i for i in range(IMG_PER_CORE)]
        coef_c = np.zeros((6, 48), np.float32)
        # mask (48, 64): cols 0:3 = W img i (softmax weights), cols
        # 32:35 = S img i (ones); other cols are dummy ones so every psum
        # row gets initialized (the same mask serves both px-halves)
        mask = np.ones((48, 64), np.float32)
        mask[:, 0:3] = 0.0
        mask[:, 32:35] = 0.0
        for i, a in enumerate(imgs):
            coef_c[:, 16 * i : 16 * i + K] = coef[a].T
            mask[16 * i : 16 * i + K, i] = w[a]
            mask[16 * i : 16 * i + K, 32 + i] = 1.0
        in_maps.append(
            {
                "bc0": np.ascontiguousarray(
                    np.concatenate([basis[:, 0:512], coef_c], axis=1)),
                "bc1": np.ascontiguousarray(basis[:, 512:1024]),
                "mask": mask.astype(ml_dtypes.bfloat16),
                "utw": np.ascontiguousarray(uT),
                "uth": uth,
            }
        )
    return in_maps


# ----------------------------------------------------------------------------
# Bass kernel
# ----------------------------------------------------------------------------

_NC_CACHE = {}


def _build_nc():
    if "nc" in _NC_CACHE:
        return _NC_CACHE["nc"]

    import concourse.bacc as bacc
    import concourse.mybir as mybir
    import concourse.tile as tile

    f32 = mybir.dt.float32
    f32r = mybir.dt.float32r
    bf16 = mybir.dt.bfloat16
    nc = bacc.Bacc("TRN2", target_bir_lowering=False, debug=False,
                   enable_asserts=False)

    bc0_d = nc.dram_tensor("bc0", (6, 560), f32r,
                           kind="ExternalInput").ap()
    bc1_d = nc.dram_tensor("bc1", (6, 512), f32r,
                           kind="ExternalInput").ap()
    mask_d = nc.dram_tensor("mask", (48, 64), bf16, kind="ExternalInput").ap()
    utw_d = nc.dram_tensor("utw", (NC, 256), f32r, kind="ExternalInput").ap()
    uth_d = nc.dram_tensor("uth", (96, 256), f32r, kind="ExternalInput").ap()
    out_d = nc.dram_tensor("out", (IMG_PER_CORE, 2, 128, W), f32,
                           kind="ExternalOutput").ap()

    EXP = mybir.ActivationFunctionType.Exp

    with tile.TileContext(nc) as tc:
        with ExitStack() as ctx:
            const_pool = ctx.enter_context(tc.tile_pool(name="const", bufs=1))
            pq_pool = ctx.enter_context(
                tc.tile_pool(name="pq", bufs=2, space="PSUM"))
            pr_pool = ctx.enter_context(
                tc.tile_pool(name="pr", bufs=1, space="PSUM"))
            py_pool = ctx.enter_context(
                tc.tile_pool(name="py", bufs=1, space="PSUM"))
            po_pool = ctx.enter_context(
                tc.tile_pool(name="po", bufs=3, space="PSUM"))
            sb_pool = ctx.enter_context(tc.tile_pool(name="sb", bufs=1))
            o_pool = ctx.enter_context(tc.tile_pool(name="o", bufs=3))
            dram_pool = ctx.enter_context(
                tc.tile_pool(name="dstage", bufs=1, space="DRAM"))

            # Warm-up matmuls: ramp the PE p-state during the input DMAs
            warm_sb = const_pool.tile([128, 512], bf16)
            nc.gpsimd.memset(warm_sb[:], 0.0)
            warm_ps = po_pool.tile([128, 512], f32, tag="po")
            for _ in range(N_WARM):
                nc.tensor.matmul(warm_ps[:], warm_sb[:, 0:128], warm_sb[:],
                                 start=True, stop=True)

            # Input DMAs (sync + scalar hwdge queues)
            bc0_sb = const_pool.tile([6, 560], f32r)
            bc1_sb = const_pool.tile([6, 512], f32r)
            mask_sb = const_pool.tile([48, 64], bf16)
            utw_sb = const_pool.tile([NC, 256], f32r)
            uth_sb = const_pool.tile([96, 256], f32r)
            nc.sync.dma_start(bc0_sb[:], bc0_d[:])
            nc.sync.dma_start(bc1_sb[:], bc1_d[:])
            nc.scalar.dma_start(mask_sb[:], mask_d[:])
            nc.scalar.dma_start(utw_sb[:], utw_d[:])
            nc.scalar.dma_start(uth_sb[:], uth_d[:])
            coef_sb = bc0_sb[:, 512:560]
            basis_h = [bc0_sb[:, 0:512], bc1_sb[:, 0:512]]

            # 1-2. q-matmul + exp, two independent 512-px halves (separate
            # psum tiles so q-mm h1 isn't false-serialized behind exp h0)
            e_sb = sb_pool.tile([48, CPX], bf16, tag="e")
            for hf in range(2):
                ps_q = pq_pool.tile([48, 512], f32, tag="pq",
                                    name=f"ps_q{hf}")
                sl = slice(512 * hf, 512 * (hf + 1))
                nc.tensor.matmul(ps_q[:], coef_sb, basis_h[hf],
                                 start=True, stop=True)
                nc.scalar.activation(e_sb[:, sl], ps_q[:], EXP)

            # 3. reduction matmuls per px-half into one packed psum tile:
            # W-h0 rows 0:32, W-h1 32:64, S-h0 64:96, S-h1 96:128
            # (img rows are the first 3 of each 32-block, rest dummy)
            ps_ws = pr_pool.tile([128, 512], f32)
            for hf in range(2):
                rhs = e_sb[:, 512 * hf : 512 * (hf + 1)]
                nc.tensor.matmul(
                    ps_ws[32 * hf : 32 * hf + 32, :], mask_sb[:, 0:32],
                    rhs, start=True, stop=True, tile_position=(0, 32 * hf),
                )
                nc.tensor.matmul(
                    ps_ws[64 + 32 * hf : 96 + 32 * hf, :], mask_sb[:, 32:64],
                    rhs, start=True, stop=True,
                    tile_position=(0, 64 + 32 * hf),
                )

            # 4-5. normalize y = W * (1/S): full-width recip (the custom
            # reciprocal DVE op corrupts data at a nonzero partition base),
            # then one shifted-window mul (row p pairs W at p with 1/S at
            # p+32; rows 32:64 compute garbage that is never read)
            rT = sb_pool.tile([128, 512], f32, tag="rT")
            y_sb = sb_pool.tile([128, 512], f32, tag="y")
            nc.vector.reciprocal_approx_fast(rT[:], ps_ws[:])
            nc.vector.tensor_mul(y_sb[0:64, :], ps_ws[0:64, :], rT[64:128, :])

            # dependency-anchored fillers: keep the PE p-state ramped
            # through the transpose stall (they read e_sb, so the scheduler
            # cannot hoist them before the exps)
            for _ in range(6):
                nc.tensor.matmul(warm_ps[:], e_sb[:, 0:128],
                                 e_sb[:, 0:512], start=True, stop=True)

            # 6-11. per-image pipeline: transpose DMA -> w-upsample matmul
            # -> psum->sbuf copy -> 2 h-upsample matmuls -> copy -> out-DMA.
            # Each image's chain starts as soon as its own T-DMA lands.
            ty_sb = sb_pool.tile([NC, 96], f32r, tag="ty")
            ps_y2a = py_pool.tile([NC, 512], f32, tag="y2pa")
            ps_y2b = py_pool.tile([NC, 256], f32, tag="y2pb")
            ps_y2 = [ps_y2a[:, 0:256], ps_y2a[:, 256:512], ps_y2b[:]]
            y2_sb = sb_pool.tile([96, 256], f32r, tag="y2")
            t_engines = [nc.sync, nc.scalar, nc.sync]
            y2_engines = [nc.scalar, nc.scalar, nc.scalar]
            copy_engines = [nc.vector, nc.vector, nc.scalar]
            dma_engines = [nc.sync, nc.scalar, nc.sync]
            for i in range(IMG_PER_CORE):
                # transpose: 2-partition strided source {i, 32+i}, dst ty
                # (32=cw, 32=ch); SBUF APs keep the partition dim leading
                srcp = y_sb[:, :].bitcast(f32r)[i : i + 33 : 32, :]
                srcp = srcp.rearrange("h (cw ch) -> h cw ch", cw=16)
                t_engines[i].dma_start(
                    ty_sb[:, 32 * i : 32 * i + NC], srcp)

                nc.tensor.matmul(
                    ps_y2[i], ty_sb[:, 32 * i : 32 * i + NC], utw_sb[:],
                    start=True, stop=True,
                )
                eng = y2_engines[i]
                if eng is nc.scalar:
                    eng.copy(y2_sb[32 * i : 32 * i + NC, :], ps_y2[i])
                else:
                    eng.tensor_copy(y2_sb[32 * i : 32 * i + NC, :],
                                    ps_y2[i])

                ps_o = po_pool.tile([128, 512], f32, tag="po",
                                    name=f"po_{i}")
                o_sb = o_pool.tile([128, 512], f32, name=f"o_{i}")
                for hh in range(2):
                    nc.tensor.matmul(
                        ps_o[:, 256 * hh : 256 * (hh + 1)],
                        uth_sb[32 * i : 32 * i + NC,
                               128 * hh : 128 * (hh + 1)],
                        y2_sb[32 * i : 32 * i + NC, :],
                        start=True, stop=True,
                    )
                eng = copy_engines[i]
                if eng is nc.scalar:
                    eng.copy(o_sb[:], ps_o[:])
                else:
                    eng.tensor_copy(o_sb[:], ps_o[:])
                if i < 2:
                    # one DMA per image: o_sb cols are (hh, w); the DRAM
                    # side reorders to out[i][hh][p][w]
                    dst = out_d[i].rearrange("hh p w -> p hh w")
                    dma_engines[i].dma_start(dst, o_sb[:])
                else:
                    # last image: two half-DMAs on both queues so the final
                    # 256KB transfer runs at double width
                    nc.sync.dma_start(out_d[i, 0], o_sb[:, 0:256])
                    nc.scalar.dma_start(out_d[i, 1], o_sb[:, 256:512])

    nc.compile()
    _NC_CACHE["nc"] = nc
    return nc


def _run(in_maps, **spmd_kwargs):
    from concourse.bass_utils import run_bass_kernel_spmd

    nc = _build_nc()
    return run_bass_kernel_spmd(
        nc, in_maps, core_ids=list(range(N_CORES)), **spmd_kwargs
    )


def _assemble(results):
    """results: 8 dicts with 'out' (3, 2, 128, 256) -> (8, 3, 256, 256)."""
    full = np.empty((A, H, W), dtype=np.float32)
    for c, res in enumerate(results):
        full[3 * c : 3 * c + 3] = res["out"].reshape(3, H, W)
    return full.reshape(8, 3, H, W)


def kernel(params, height, width):
    assert int(height) == H and int(width) == W
    in_maps = _host_inputs(params)
    res = _run(in_maps)
    return _assemble(res.results)


if __name__ == "__main__":
    params = np.random.RandomState(0).randn(8, 3, 7 * K).astype(np.float32)
    out = kernel(params, 256, 256)
    print("kernel ran, out", out.shape, out.dtype, np.isnan(out).sum())
